# revision 35
# baseline (speedup 1.0000x reference)
"""Trainium2 Bass kernel for nn_AllAtomPathAggregator.

Strategy (8 NeuronCores, SPMD):
  - The per-(path, energy) MLP `g_elem` depends on paths only through the
    element pair (z_j, z_k) -> dedupe it into a class table over the
    100x100 z-grid.  Each core builds the table for ALL 10000 classes but
    only ITS slice of 10 (of 80) energy channels (table sharded over e).
  - Paths are sorted by center atom on the host (index plumbing) and padded
    into 8 windows of 128 atoms x 5632 path slots, so the segment-sum
    becomes windowed one-hot matmuls accumulating in PSUM.
  - g_geom (per-path MLP, no e-dim) is path-sharded across cores; the
    weighted per-path vector m = cutoff_w * g_geom is AllGathered.
  - Every core gathers table rows for all paths (dma_gather, fp16 768B
    rows), multiplies by m, scatter-adds into its (1024, 10e, 32) agg via
    one-hot matmuls, normalizes, applies the output MLP and writes its
    (1024, 10e, 128) slice.  Host concatenates the 8 e-slices.
"""
import os
import sys

sys.path.insert(0, "/opt/trn_rl_repo")
sys.path.insert(0, "/opt/trn_rl_repo/concourse")

import numpy as np

import concourse.bass as bass
import concourse.bacc as bacc
import concourse.mybir as mybir
from concourse.tile import TileContext

F32 = mybir.dt.float32
F16 = mybir.dt.float16
I16 = mybir.dt.int16
AF = mybir.ActivationFunctionType
ALU = mybir.AluOpType

NCORES = 8
BN = 1024            # atoms (16*64)
P = 40000            # paths
NE = 80              # energy channels
NEL = NE // NCORES   # 10 per core
NZ = 100             # z values 1..100
NCLS = NZ * NZ       # 10000
NCLS_PAD = 10240     # 5 chunks of 2048
CW = 2048            # class chunk
ATOM_DIM = 128
SCAT = 32            # scatter_dim
PHID = 128           # pair_hidden
GHID = 256           # geom_hidden
ODIM = 128           # out_dim
RBF = 32
CUTOFF = 5.0
EPS = 1e-8

SLOTS = 5632                 # padded path slots per 128-atom window
NWIN = BN // 128             # 8 windows
PPAD = SLOTS * NWIN          # 45056
CHUNKS = PPAD // 128         # 352 path chunks
WCH = SLOTS // 128           # 44 chunks per window / per core shard
GBLK = 512                   # g_geom path block
NBLK = SLOTS // GBLK         # 11 blocks per core
SG = 8                       # chunks per gather super-group (1024 paths)
NSG = CHUNKS // SG           # 44
GROW = 384                   # fp16 elems per table row (320 used + pad) = 768B

RBF_W = CUTOFF / (RBF - 1)
RBF_COEF = -0.5 / (RBF_W * RBF_W)

# 0.5*(cos(pi*sqrt(u)/5)+1) on u = r^2 in [0, 36] (deg-9 poly, err ~1e-7)
_u = np.linspace(0.0, 36.0, 4000)
_y = 0.5 * (np.cos(np.pi * np.sqrt(_u) / CUTOFF) + 1.0)
COS_COEF = np.polynomial.chebyshev.Chebyshev.fit(_u, _y, 9).convert(
    kind=np.polynomial.polynomial.Polynomial).coef.astype(np.float64)


def build_program():
    nc = bacc.Bacc(None, target_bir_lowering=False)
    d = nc.dram_tensor
    # ---------- external inputs ----------
    h16 = d("h16", [BN, ATOM_DIM], F16, kind="ExternalInput")
    zET = d("zET", [32, NZ + 1], F16, kind="ExternalInput")
    eFT = d("eFT", [32, NEL], F16, kind="ExternalInput")
    peW1j = d("peW1j", [32, PHID], F16, kind="ExternalInput")
    peW1k = d("peW1k", [32, PHID], F16, kind="ExternalInput")
    peW1e = d("peW1e", [32, PHID], F16, kind="ExternalInput")
    peW2 = d("peW2", [PHID, PHID], F16, kind="ExternalInput")
    peW3 = d("peW3", [PHID, SCAT], F16, kind="ExternalInput")
    peB1 = d("peB1", [PHID, 1], F32, kind="ExternalInput")
    peB2 = d("peB2", [PHID, 1], F32, kind="ExternalInput")
    peB3 = d("peB3", [128, 1], F32, kind="ExternalInput")
    gW1hj = d("gW1hj", [128, GHID], F16, kind="ExternalInput")
    gW1hk = d("gW1hk", [128, GHID], F16, kind="ExternalInput")
    gW1f = d("gW1f", [97, GHID], F16, kind="ExternalInput")
    gW2a = d("gW2a", [128, GHID], F16, kind="ExternalInput")
    gW2b = d("gW2b", [128, GHID], F16, kind="ExternalInput")
    gW3a = d("gW3a", [128, SCAT], F16, kind="ExternalInput")
    gW3b = d("gW3b", [128, SCAT], F16, kind="ExternalInput")
    gB1 = d("gB1", [128, 2], F32, kind="ExternalInput")
    gB2 = d("gB2", [128, 2], F32, kind="ExternalInput")
    gB3t = d("gB3t", [128, SCAT], F32, kind="ExternalInput")
    oW1 = d("oW1", [SCAT, GHID], F16, kind="ExternalInput")
    oW2a = d("oW2a", [128, ODIM], F16, kind="ExternalInput")
    oW2b = d("oW2b", [128, ODIM], F16, kind="ExternalInput")
    oB1 = d("oB1", [128, 2], F32, kind="ExternalInput")
    oB2 = d("oB2", [128, 1], F32, kind="ExternalInput")
    # per-core path shard data
    rall = d("rall", [128, 3 * WCH], F32, kind="ExternalInput")   # rj|rk|rjk
    cang = d("cang", [128, WCH], F32, kind="ExternalInput")
    idxj = d("idxj", [128, NBLK * 32], I16, kind="ExternalInput")
    idxk = d("idxk", [128, NBLK * 32], I16, kind="ExternalInput")
    # replicated path data (all 45056 slots)
    ctrd = d("ctrd", [128, CHUNKS], F32, kind="ExternalInput")
    idxc = d("idxc", [128, NSG * 64], I16, kind="ExternalInput")
    # ---------- output ----------
    out = d("out", [BN, NEL, ODIM], F32, kind="ExternalOutput")
    # ---------- internal DRAM ----------
    G = d("G", [NCLS_PAD, GROW], F16)
    m_self = d("m_self", [128, WCH * 33], F16)
    m_shared = d("m_shared", [128 * NCORES, WCH * 33], F16, addr_space="Shared")

    with TileContext(nc) as tc:
        with tc.tile_pool(name="c1", bufs=1) as c1, \
             tc.tile_pool(name="wk", bufs=2) as wk, \
             tc.tile_pool(name="gb", bufs=3) as gpool, \
             tc.tile_pool(name="pbig", bufs=2, space="PSUM") as pbig, \
             tc.tile_pool(name="ptr", bufs=2, space="PSUM") as ptr, \
             tc.tile_pool(name="pagg", bufs=2, space="PSUM") as pagg:
            pl3 = ptr

            # ================= setup =================
            def load(t, dram, dt):
                tt = c1.tile(list(dram.shape), dt, tag=t)
                nc.sync.dma_start(out=tt[:], in_=dram[:])
                return tt

            zETt = load("zET", zET, F16)
            eFTt = load("eFT", eFT, F16)
            peW1jt = load("peW1j", peW1j, F16)
            peW1kt = load("peW1k", peW1k, F16)
            peW1et = load("peW1e", peW1e, F16)
            peW2t = load("peW2", peW2, F16)
            peW3t = load("peW3", peW3, F16)
            peB1t = load("peB1", peB1, F32)
            peB2t = load("peB2", peB2, F32)
            peB3t = load("peB3", peB3, F32)
            gW1hjt = load("gW1hj", gW1hj, F16)
            gW1hkt = load("gW1hk", gW1hk, F16)
            gW1ft = load("gW1f", gW1f, F16)
            gW2at = load("gW2a", gW2a, F16)
            gW2bt = load("gW2b", gW2b, F16)
            gW3at = load("gW3a", gW3a, F16)
            gW3bt = load("gW3b", gW3b, F16)
            gB1t = load("gB1", gB1, F32)
            gB2t = load("gB2", gB2, F32)
            gB3tt = load("gB3t", gB3t, F32)
            oW1t = load("oW1", oW1, F16)
            oW2at = load("oW2a", oW2a, F16)
            oW2bt = load("oW2b", oW2b, F16)
            oB1t = load("oB1", oB1, F32)
            oB2t = load("oB2", oB2, F32)
            rallt = load("rall", rall, F32)
            cangt = load("cang", cang, F32)
            idxjt = load("idxj", idxj, I16)
            idxkt = load("idxk", idxk, I16)
            ctrt = load("ctrd", ctrd, F32)
            idxct = load("idxc", idxc, I16)

            # identity (fp16) for PE transposes + window iota tiles (f32)
            iota_row = c1.tile([128, 128], F32, tag="iota_row")
            nc.gpsimd.iota(iota_row[:], pattern=[[1, 128]], base=0,
                           channel_multiplier=0,
                           allow_small_or_imprecise_dtypes=True)
            iota_col = c1.tile([128, 1], F32, tag="iota_col")
            nc.gpsimd.iota(iota_col[:], pattern=[[0, 1]], base=0,
                           channel_multiplier=1,
                           allow_small_or_imprecise_dtypes=True)
            ident = c1.tile([128, 128], F16, tag="ident")
            nc.vector.tensor_tensor(ident[:], iota_row[:],
                                    iota_col[:].broadcast_to([128, 128]),
                                    ALU.is_equal)
            iwtmp = c1.tile([128, 128], F32, tag="iwtmp")
            iwin = []
            for w in range(NWIN):
                nc.gpsimd.iota(iwtmp[:], pattern=[[1, 128]], base=128 * w,
                               channel_multiplier=0,
                               allow_small_or_imprecise_dtypes=True)
                t = c1.tile([128, 128], F16, tag=f"iwin{w}")
                nc.vector.tensor_copy(t[:], iwtmp[:])
                iwin.append(t)
            # rbf centers tile (128, 32) f32: k * CUTOFF/(RBF-1)
            cent = c1.tile([128, RBF], F32, tag="cent")
            nc.gpsimd.iota(cent[:], pattern=[[1, RBF]], base=0,
                           channel_multiplier=0,
                           allow_small_or_imprecise_dtypes=True)
            nc.vector.tensor_scalar_mul(cent[:], cent[:], float(RBF_W))

            # ---- tiny matmuls: tjT/tkT (128,100) and vT (128, NEL) ----
            tjp = pl3.tile([128, NZ], F32, tag="tr")
            nc.tensor.matmul(tjp[:], peW1jt[:], zETt[:, 1:NZ + 1],
                             start=True, stop=True)
            tjTs = c1.tile([128, NZ], F32, tag="tjT")
            nc.vector.tensor_scalar_add(tjTs[:], tjp[:], peB1t[:])
            tkp = pl3.tile([128, NZ], F32, tag="tr")
            nc.tensor.matmul(tkp[:], peW1kt[:], zETt[:, 1:NZ + 1],
                             start=True, stop=True)
            tkTs = c1.tile([128, NZ], F32, tag="tkT")
            nc.vector.tensor_copy(tkTs[:], tkp[:])
            vp = pl3.tile([128, NEL], F32, tag="tr")
            nc.tensor.matmul(vp[:], peW1et[:], eFTt[:],
                             start=True, stop=True)
            vTs = c1.tile([128, NEL], F32, tag="vT")
            nc.vector.tensor_copy(vTs[:], vp[:])

            # ---- u grid (128, NCLS_PAD) fp16 ----
            ugrid = c1.tile([128, NCLS_PAD], F16, tag="ugrid")
            nc.vector.memset(ugrid[:, NCLS:NCLS_PAD], 0.0)
            nc.vector.tensor_tensor(
                ugrid[:, 0:NCLS].rearrange("p (a b) -> p a b", b=NZ),
                tjTs[:].unsqueeze(2).broadcast_to([128, NZ, NZ]),
                tkTs[:].unsqueeze(1).broadcast_to([128, NZ, NZ]),
                ALU.add)

            # ================= phase A: g_geom + m for own shard ============
            # rbf features + cutoff weights for own 5632 paths
            rmin = wk.tile([128, 3 * WCH], F32, tag="rmin")
            nc.vector.tensor_scalar_min(rmin[:], rallt[:], float(CUTOFF))
            # cutoff polynomial in u = r*r
            upoly = wk.tile([128, 3 * WCH], F32, tag="upoly")
            nc.vector.tensor_tensor(upoly[:], rallt[:], rallt[:], ALU.mult)
            y0 = wk.tile([128, 3 * WCH], F32, tag="y0")
            y1 = wk.tile([128, 3 * WCH], F32, tag="y1")
            nc.vector.memset(y0[:], float(COS_COEF[9]))
            cur, nxt = y0, y1
            for k in range(8, -1, -1):
                nc.vector.tensor_tensor(nxt[:], cur[:], upoly[:], ALU.mult)
                nc.vector.tensor_scalar_add(nxt[:], nxt[:], float(COS_COEF[k]))
                cur, nxt = nxt, cur
            mask = wk.tile([128, 3 * WCH], F32, tag="mask")
            nc.vector.tensor_scalar(mask[:], rallt[:], float(CUTOFF), None,
                                    ALU.is_lt)
            nc.vector.tensor_tensor(cur[:], cur[:], mask[:], ALU.mult)
            wpr = c1.tile([128, WCH], F32, tag="wpr")
            nc.vector.tensor_tensor(wpr[:], cur[:, 0:WCH], cur[:, WCH:2 * WCH],
                                    ALU.mult)
            nc.vector.tensor_tensor(wpr[:], wpr[:], cur[:, 2 * WCH:3 * WCH],
                                    ALU.mult)

            # rbf feature pack, path-major: (128, WCH, 112)
            feat = c1.tile([128, WCH, 112], F16, tag="feat")
            for kind in range(3):
                rsl = rmin[:, kind * WCH:(kind + 1) * WCH]
                dmt = wk.tile([128, WCH, RBF], F32, tag="dmt")
                nc.vector.tensor_tensor(
                    dmt[:], cent[:].unsqueeze(1).broadcast_to([128, WCH, RBF]),
                    rsl.unsqueeze(2).broadcast_to([128, WCH, RBF]),
                    ALU.subtract)
                nc.vector.tensor_tensor(dmt[:], dmt[:], dmt[:], ALU.mult)
                nc.scalar.activation(feat[:, :, kind * RBF:(kind + 1) * RBF],
                                     dmt[:], AF.Exp, scale=float(RBF_COEF))
            nc.vector.tensor_copy(feat[:, :, 96:97], cangt[:].unsqueeze(2))

            m_all = c1.tile([128, WCH, 33], F16, tag="m_all")
            nc.vector.tensor_copy(m_all[:, :, 32:33], wpr[:].unsqueeze(2))

            for b in range(NBLK):
                hjT = gpool.tile([128, 1, GBLK], F16, tag="hjT")
                nc.gpsimd.dma_gather(hjT[:], h16[:],
                                     idxjt[:, 32 * b:32 * (b + 1)],
                                     GBLK, GBLK, ATOM_DIM, transpose=True)
                hkT = gpool.tile([128, 1, GBLK], F16, tag="hkT")
                nc.gpsimd.dma_gather(hkT[:], h16[:],
                                     idxkt[:, 32 * b:32 * (b + 1)],
                                     GBLK, GBLK, ATOM_DIM, transpose=True)
                fT = wk.tile([97, GBLK], F16, tag="fT")
                for c in range(4):
                    tp = pagg.tile([97, 128], F16, tag="agg")
                    nc.tensor.transpose(tp[:], feat[:, 4 * b + c, 0:97],
                                        ident[:])
                    nc.vector.tensor_copy(fT[:, 128 * c:128 * (c + 1)], tp[:])
                # L1: H1T (256, 512) = silu(W1^T X^T + b1)
                h1a = wk.tile([128, GBLK], F16, tag="h1a")
                h1b = wk.tile([128, GBLK], F16, tag="h1b")
                for half, h1 in ((0, h1a), (1, h1b)):
                    ps = pagg.tile([128, GBLK], F32, tag="agg")
                    sl = slice(128 * half, 128 * (half + 1))
                    nc.tensor.matmul(ps[:], gW1hjt[:, sl], hjT[:, 0, :], start=True,
                                     stop=False)
                    nc.tensor.matmul(ps[:], gW1hkt[:, sl], hkT[:, 0, :], start=False,
                                     stop=False)
                    nc.tensor.matmul(ps[:], gW1ft[:, sl], fT[:], start=False,
                                     stop=True)
                    nc.scalar.activation(h1[:], ps[:], AF.Silu,
                                         bias=gB1t[:, half:half + 1])
                # L2
                h2a = wk.tile([128, GBLK], F16, tag="h2a")
                h2b = wk.tile([128, GBLK], F16, tag="h2b")
                for half, h2 in ((0, h2a), (1, h2b)):
                    ps = pagg.tile([128, GBLK], F32, tag="agg")
                    sl = slice(128 * half, 128 * (half + 1))
                    nc.tensor.matmul(ps[:], gW2at[:, sl], h1a[:], start=True,
                                     stop=False)
                    nc.tensor.matmul(ps[:], gW2bt[:, sl], h1b[:], start=False,
                                     stop=True)
                    nc.scalar.activation(h2[:], ps[:], AF.Silu,
                                         bias=gB2t[:, half:half + 1])
                # L3 path-major + bias + cutoff weight -> m_all
                for c in range(4):
                    cc = 4 * b + c
                    pm = pagg.tile([128, SCAT], F32, tag="agg")
                    sl = slice(128 * c, 128 * (c + 1))
                    nc.tensor.matmul(pm[:], h2a[:, sl], gW3at[:], start=True,
                                     stop=False)
                    nc.tensor.matmul(pm[:], h2b[:, sl], gW3bt[:], start=False,
                                     stop=True)
                    t32 = wk.tile([128, SCAT], F32, tag="t32")
                    nc.vector.tensor_tensor(t32[:], pm[:], gB3tt[:], ALU.add)
                    nc.vector.tensor_scalar_mul(m_all[:, cc, 0:SCAT], t32[:],
                                                wpr[:, cc:cc + 1])
            nc.sync.dma_start(out=m_self[:], in_=m_all[:])
            # ================= phase T: class table ====================
            # e-groups stacked 4-wide on partitions for the L3 drain
            EGROUPS = [(0, 4), (4, 4), (8, 2)]
            for e0, Q in EGROUPS:
                for cc in range(NCLS_PAD // CW):
                    usl = ugrid[:, CW * cc:CW * (cc + 1)]
                    h2s = []
                    for q in range(Q):
                        h1 = wk.tile([128, CW], F16, tag="th1")
                        nc.scalar.activation(h1[:], usl, AF.Silu,
                                             bias=vTs[:, e0 + q:e0 + q + 1])
                        h2 = c1.tile([128, CW], F16, tag=f"th2q{q}")
                        for half in range(2):
                            ps = pbig.tile([128, 1024], F32, tag="big")
                            for p2 in range(2):
                                sl = slice(1024 * half + 512 * p2,
                                           1024 * half + 512 * (p2 + 1))
                                nc.tensor.matmul(ps[:, 512 * p2:512 * (p2 + 1)],
                                                 peW2t[:], h1[:, sl],
                                                 start=True, stop=True)
                            nc.scalar.activation(
                                h2[:, 1024 * half:1024 * (half + 1)], ps[:],
                                AF.Silu, bias=peB2t[:])
                        h2s.append(h2)
                    gbt4 = wk.tile([128, CW], F16, tag="tgb")
                    for piece in range(4):
                        ps3 = pl3.tile([128, 512], F32, tag="tr")
                        sl = slice(512 * piece, 512 * (piece + 1))
                        for q in range(Q):
                            nc.tensor.matmul(ps3[32 * q:32 * (q + 1), :],
                                             peW3t[:], h2s[q][:, sl],
                                             start=True, stop=True,
                                             tile_position=(0, 32 * q),
                                             skip_group_check=True)
                        nc.vector.tensor_scalar_add(
                            gbt4[0:32 * Q, sl], ps3[0:32 * Q, :], peB3t[0:32 * Q, :])
                    for q in range(Q):
                        trb = ptr.tile([128, 512], F16, tag="tr")
                        qq = slice(32 * q, 32 * (q + 1))
                        for t in range(16):
                            nc.tensor.transpose(
                                trb[:, 32 * t:32 * (t + 1)],
                                gbt4[qq, 128 * t:128 * (t + 1)],
                                ident[qq, qq],
                                tile_position=(32 * q, 0))
                        gst = wk.tile([128, 512], F16, tag="gst")
                        nc.vector.tensor_copy(gst[:], trb[:])
                        e = e0 + q
                        nc.sync.dma_start(
                            out=G[CW * cc:CW * (cc + 1),
                                  32 * e:32 * (e + 1)].rearrange(
                                      "(t p) s -> p t s", p=128),
                            in_=gst[:].rearrange("p (t s) -> p t s", s=32))

            nc.gpsimd.collective_compute(
                "AllGather", ALU.bypass,
                replica_groups=[list(range(NCORES))],
                ins=[m_self[:]], outs=[m_shared[:]])
            m_full = c1.tile([128, CHUNKS, 33], F16, tag="m_full")
            nc.gpsimd.dma_start(
                out=m_full[:].rearrange("p (r c) s -> p r c s", r=NCORES),
                in_=m_shared.rearrange("(r p) (c s) -> p r c s", p=128, s=33))

            # ============ phase S + O: gather/contrib/scatter + out MLP ====
            tc.tile_set_cur_wait(0.185)
            aggps = [None] * NWIN
            ohts = {}
            for sg in range(NSG):
                gbuf = gpool.tile([128, SG, GROW], F16, tag="gbuf")
                nc.gpsimd.dma_gather(gbuf[:], G[:],
                                     idxct[:, 64 * sg:64 * (sg + 1)],
                                     SG * 128, SG * 128, GROW)
                contrib = wk.tile([128, SG, 354], F16, tag="contrib")
                nc.vector.tensor_tensor(
                    contrib[:, :, 0:320].rearrange(
                        "p c (e s) -> p c e s", s=32),
                    gbuf[:, :, 0:320].rearrange("p c (e s) -> p c e s", s=32),
                    m_full[:, SG * sg:SG * (sg + 1), 0:SCAT].unsqueeze(
                        2).broadcast_to([128, SG, NEL, SCAT]),
                    ALU.mult)
                nc.scalar.activation(
                    contrib[:, :, 321:354],
                    m_full[:, SG * sg:SG * (sg + 1), 0:33], AF.Copy)
                for c in range(SG):
                    C = SG * sg + c
                    w = C // WCH
                    t = C % WCH
                    oht = wk.tile([128, 128], F16, tag="oht")
                    nc.vector.tensor_scalar(oht[:], iwin[w][:],
                                            ctrt[:, C:C + 1], None,
                                            ALU.is_equal)
                    oh = oht[:]
                    if t == 0:
                        agg_t = pagg.tile([128, 354], F32, tag="agg")
                        aggps[w] = agg_t
                    nc.tensor.matmul(aggps[w][:], oh, contrib[:, c, :],
                                     start=(t == 0), stop=(t == WCH - 1),
                                     skip_group_check=True)
                    if t == WCH - 1:
                        _emit_window_out(nc, wk, pbig, pl3, ptr, aggps, w,
                                         ident, oW1t, oW2at, oW2bt, oB1t,
                                         oB2t, out)
    nc.compile()
    return nc


def _emit_window_out(nc, wk, pbig, pl3, ptr, aggps, w, ident, oW1t, oW2at,
                     oW2bt, oB1t, oB2t, out):
    """Normalize window w's agg and run the output MLP, write DRAM."""
    agg = aggps[w]
    nrm = wk.tile([128, 1], F32, tag="nrm")
    nc.vector.tensor_scalar_max(nrm[:], agg[:, 353:354], float(EPS))
    rin = wk.tile([128, 1], F32, tag="rin")
    nc.vector.reciprocal(rin[:], nrm[:])
    aggn = wk.tile([128, 320], F16, tag="aggn")
    nc.vector.tensor_scalar_mul(aggn[:], agg[:, 0:320], rin[:])
    slab = wk.tile([128, NEL, ODIM], F32, tag="slab")
    EG = 5
    for g in range(NEL // EG):
        xts = wk.tile([SCAT, EG, 128], F16, tag="xts")
        for q in range(EG):
            e = EG * g + q
            xt = ptr.tile([SCAT, 128], F16, tag="tr")
            nc.tensor.transpose(xt[:], aggn[:, 32 * e:32 * (e + 1)], ident[:])
            nc.scalar.activation(xts[:, q, :], xt[:], AF.Copy)
        h1a = wk.tile([128, EG * 128], F16, tag="oh1a")
        h1b = wk.tile([128, EG * 128], F16, tag="oh1b")
        for half, h1 in ((0, h1a), (1, h1b)):
            ps = pbig.tile([128, EG * 128], F32, tag="big")
            sl = slice(128 * half, 128 * (half + 1))
            xr = xts[:].rearrange("p q a -> p (q a)")
            nc.tensor.matmul(ps[:, 0:512], oW1t[:, sl], xr[:, 0:512],
                             start=True, stop=True, skip_group_check=True)
            nc.tensor.matmul(ps[:, 512:640], oW1t[:, sl], xr[:, 512:640],
                             start=True, stop=True, skip_group_check=True)
            nc.scalar.activation(h1[:], ps[:], AF.Silu,
                                 bias=oB1t[:, half:half + 1])
        po = pbig.tile([128, EG * 128], F32, tag="big")
        nc.tensor.matmul(po[:, 0:512], oW2at[:], h1a[:, 0:512],
                         start=True, stop=False, skip_group_check=True)
        nc.tensor.matmul(po[:, 0:512], oW2bt[:], h1b[:, 0:512],
                         start=False, stop=True, skip_group_check=True)
        nc.tensor.matmul(po[:, 512:640], oW2at[:], h1a[:, 512:640],
                         start=True, stop=False, skip_group_check=True)
        nc.tensor.matmul(po[:, 512:640], oW2bt[:], h1b[:, 512:640],
                         start=False, stop=True, skip_group_check=True)
        ot16 = wk.tile([128, EG * 128], F16, tag="ot16")
        nc.vector.tensor_scalar_add(ot16[:], po[:], oB2t[:])
        for q in range(EG):
            e = EG * g + q
            tro = ptr.tile([128, 128], F16, tag="tr")
            nc.tensor.transpose(tro[:], ot16[:, 128 * q:128 * (q + 1)],
                                ident[:])
            nc.scalar.activation(slab[:, e, :], tro[:], AF.Copy)
    nc.sync.dma_start(out=out[128 * w:128 * (w + 1)], in_=slab[:])


# =================== host side ===================
_PROG = None
LAST_RESULT = None


def _wrap_idx(a, gs):
    """dma_gather index layout: groups of gs, 16-wrap, replicated x8."""
    a = np.asarray(a, np.int64)
    ng = len(a) // gs
    blk = a.reshape(ng, gs // 16, 16)           # [g, s, p] = a[g, s*16+p]
    outc = blk.transpose(2, 0, 1).reshape(16, ng * (gs // 16))
    return np.tile(outc, (8, 1)).astype(np.int16)


def _colchunk(a):
    return np.ascontiguousarray(np.asarray(a).reshape(-1, 128).T)


def _prepare(inputs):
    h_flat = np.asarray(inputs["h_flat"], np.float32)
    z_flat = np.asarray(inputs["z_flat"], np.int32)
    e_feat = np.asarray(inputs["e_feat"], np.float32)
    pc = np.asarray(inputs["path_center"], np.int64)
    pj = np.asarray(inputs["path_j"], np.int64)
    pk = np.asarray(inputs["path_k"], np.int64)
    r0j = np.asarray(inputs["path_r0j"], np.float32)
    r0k = np.asarray(inputs["path_r0k"], np.float32)
    rjk = np.asarray(inputs["path_rjk"], np.float32)
    ca = np.asarray(inputs["path_cosangle"], np.float32)
    bsz = int(inputs["bsz"])
    n_atoms = int(inputs["n_atoms"])
    assert bsz * n_atoms == BN and pc.shape[0] == P

    # ---- sort by center, bucket into 8 windows, pad to SLOTS each ----
    order = np.argsort(pc, kind="stable")
    win = pc[order] // 128
    idx_pad = np.zeros(PPAD, np.int64)
    valid = np.zeros(PPAD, bool)
    for w in range(NWIN):
        sel = order[win == w]
        assert len(sel) <= SLOTS, f"window {w} overflow: {len(sel)}"
        idx_pad[w * SLOTS:w * SLOTS + len(sel)] = sel
        valid[w * SLOTS:w * SLOTS + len(sel)] = True

    def takef(a, fill):
        o = np.full(PPAD, fill, np.float32)
        o[valid] = a[idx_pad[valid]]
        return o

    ctr = np.where(valid, pc[idx_pad],
                   (np.arange(PPAD) // SLOTS) * 128).astype(np.float32)
    jj = np.where(valid, pj[idx_pad], 0)
    kk = np.where(valid, pk[idx_pad], 0)
    rj_p = takef(r0j, 6.0)
    rk_p = takef(r0k, 6.0)
    rjk_p = takef(rjk, 6.0)
    ca_p = takef(ca, 0.0)
    cls = ((z_flat[jj] - 1).astype(np.int64) * NZ
           + (z_flat[kk] - 1)).astype(np.int64)
    np.clip(cls, 0, NCLS - 1, out=cls)

    # ---- weights / common tensors ----
    f16 = lambda a: np.ascontiguousarray(a, np.float32).astype(np.float16)
    f32c = lambda a, shape=None: np.ascontiguousarray(
        np.asarray(a, np.float32).reshape(shape) if shape else
        np.asarray(a, np.float32))
    g_W1 = np.asarray(inputs["g_W1"], np.float32)
    g_W2 = np.asarray(inputs["g_W2"], np.float32)
    g_W3 = np.asarray(inputs["g_W3"], np.float32)
    o_W2 = np.asarray(inputs["o_W2"], np.float32)
    z_emb = np.asarray(inputs["z_emb"], np.float32)
    common = dict(
        h16=f16(h_flat),
        zET=f16(z_emb.T),
        peW1j=f16(np.asarray(inputs["pe_W1"], np.float32)[0:32]),
        peW1k=f16(np.asarray(inputs["pe_W1"], np.float32)[32:64]),
        peW1e=f16(np.asarray(inputs["pe_W1"], np.float32)[64:96]),
        peW2=f16(inputs["pe_W2"]),
        peW3=f16(inputs["pe_W3"]),
        peB1=f32c(inputs["pe_b1"], (PHID, 1)),
        peB2=f32c(inputs["pe_b2"], (PHID, 1)),
        peB3=f32c(np.tile(np.asarray(inputs["pe_b3"], np.float32), 4), (128, 1)),
        gW1hj=f16(g_W1[0:128]),
        gW1hk=f16(g_W1[128:256]),
        gW1f=f16(g_W1[256:353]),
        gW2a=f16(g_W2[0:128]),
        gW2b=f16(g_W2[128:256]),
        gW3a=f16(g_W3[0:128]),
        gW3b=f16(g_W3[128:256]),
        gB1=f32c(np.asarray(inputs["g_b1"]).reshape(2, 128).T),
        gB2=f32c(np.asarray(inputs["g_b2"]).reshape(2, 128).T),
        gB3t=f32c(np.broadcast_to(np.asarray(inputs["g_b3"], np.float32),
                                  (128, SCAT))),
        oW1=f16(inputs["o_W1"]),
        oW2a=f16(o_W2[0:128]),
        oW2b=f16(o_W2[128:256]),
        oB1=f32c(np.asarray(inputs["o_b1"]).reshape(2, 128).T),
        oB2=f32c(inputs["o_b2"], (ODIM, 1)),
        ctrd=_colchunk(ctr),
        idxc=_wrap_idx(cls, SG * 128),
    )

    in_maps = []
    for i in range(NCORES):
        sl = slice(SLOTS * i, SLOTS * (i + 1))
        rall_i = np.concatenate(
            [_colchunk(rj_p[sl]), _colchunk(rk_p[sl]), _colchunk(rjk_p[sl])],
            axis=1)
        m = dict(common)
        m.update(
            eFT=f16(e_feat[NEL * i:NEL * (i + 1)].T),
            rall=np.ascontiguousarray(rall_i),
            cang=_colchunk(ca_p[sl]),
            idxj=_wrap_idx(jj[sl], GBLK),
            idxk=_wrap_idx(kk[sl], GBLK),
        )
        in_maps.append(m)
    return in_maps


def _run_device(in_maps):
    global _PROG, LAST_RESULT
    if _PROG is None:
        _PROG = build_program()
    from concourse.bass_utils import run_bass_kernel_spmd
    res = run_bass_kernel_spmd(_PROG, in_maps, list(range(NCORES)))
    LAST_RESULT = res
    return [np.asarray(res.results[i]["out"]) for i in range(NCORES)]


def _run_subprocess(in_maps):
    """Fresh-process retry: a wedged NRT/PJRT client can only be recovered by
    a process restart (stale collective-comm state flips clean/dirty per
    process)."""
    import pickle, subprocess, tempfile, time as _time
    d = tempfile.mkdtemp()
    fin = os.path.join(d, "in.pkl")
    fout = os.path.join(d, "out.npz")
    with open(fin, "wb") as f:
        pickle.dump(in_maps, f)
    last = None
    for attempt in range(4):
        p = subprocess.run([sys.executable, os.path.abspath(__file__),
                            "--worker", fin, fout],
                           capture_output=True, text=True, timeout=1800)
        if p.returncode == 0 and os.path.exists(fout):
            z = np.load(fout)
            return [z[f"out{i}"] for i in range(NCORES)]
        last = p.stderr[-2000:]
        _time.sleep(5)
    raise RuntimeError(f"device run failed after retries: {last}")


def kernel(**inputs):
    bsz = int(inputs["bsz"])
    n_atoms = int(inputs["n_atoms"])
    in_maps = _prepare(inputs)
    try:
        outs = _run_device(in_maps)
    except Exception:
        outs = _run_subprocess(in_maps)

    full = np.empty((BN, NE, ODIM), np.float32)
    for i in range(NCORES):
        full[:, NEL * i:NEL * (i + 1), :] = outs[i]
    return full.reshape(bsz, n_atoms, NE, ODIM)


if __name__ == "__main__":
    if len(sys.argv) >= 4 and sys.argv[1] == "--worker":
        import pickle
        with open(sys.argv[2], "rb") as f:
            _maps = pickle.load(f)
        _outs = _run_device(_maps)
        np.savez(sys.argv[3], **{f"out{i}": o for i, o in enumerate(_outs)})
        print("worker ok")
    else:
        build_program()
        print("build ok")


# revision 36
# speedup vs baseline: 1.0061x; 1.0061x over previous
"""Trainium2 Bass kernel for nn_AllAtomPathAggregator.

Strategy (8 NeuronCores, SPMD):
  - The per-(path, energy) MLP `g_elem` depends on paths only through the
    element pair (z_j, z_k) -> dedupe it into a class table over the
    100x100 z-grid.  Each core builds the table for ALL 10000 classes but
    only ITS slice of 10 (of 80) energy channels (table sharded over e).
  - Paths are sorted by center atom on the host (index plumbing) and padded
    into 8 windows of 128 atoms x 5632 path slots, so the segment-sum
    becomes windowed one-hot matmuls accumulating in PSUM.
  - g_geom (per-path MLP, no e-dim) is path-sharded across cores; the
    weighted per-path vector m = cutoff_w * g_geom is AllGathered.
  - Every core gathers table rows for all paths (dma_gather, fp16 768B
    rows), multiplies by m, scatter-adds into its (1024, 10e, 32) agg via
    one-hot matmuls, normalizes, applies the output MLP and writes its
    (1024, 10e, 128) slice.  Host concatenates the 8 e-slices.
"""
import os
import sys

sys.path.insert(0, "/opt/trn_rl_repo")
sys.path.insert(0, "/opt/trn_rl_repo/concourse")

import numpy as np

import concourse.bass as bass
import concourse.bacc as bacc
import concourse.mybir as mybir
from concourse.tile import TileContext

F32 = mybir.dt.float32
F16 = mybir.dt.float16
I16 = mybir.dt.int16
AF = mybir.ActivationFunctionType
ALU = mybir.AluOpType

NCORES = 8
BN = 1024            # atoms (16*64)
P = 40000            # paths
NE = 80              # energy channels
NEL = NE // NCORES   # 10 per core
NZ = 100             # z values 1..100
NCLS = NZ * NZ       # 10000
NCLS_PAD = 10240     # 5 chunks of 2048
CW = 2048            # class chunk
ATOM_DIM = 128
SCAT = 32            # scatter_dim
PHID = 128           # pair_hidden
GHID = 256           # geom_hidden
ODIM = 128           # out_dim
RBF = 32
CUTOFF = 5.0
EPS = 1e-8

SLOTS = 5632                 # padded path slots per 128-atom window
NWIN = BN // 128             # 8 windows
PPAD = SLOTS * NWIN          # 45056
CHUNKS = PPAD // 128         # 352 path chunks
WCH = SLOTS // 128           # 44 chunks per window / per core shard
GBLK = 512                   # g_geom path block
NBLK = SLOTS // GBLK         # 11 blocks per core
SG = 8                       # chunks per gather super-group (1024 paths)
NSG = CHUNKS // SG           # 44
GROW = 384                   # fp16 elems per table row (320 used + pad) = 768B

RBF_W = CUTOFF / (RBF - 1)
RBF_COEF = -0.5 / (RBF_W * RBF_W)

# 0.5*(cos(pi*sqrt(u)/5)+1) on u = r^2 in [0, 36] (deg-9 poly, err ~1e-7)
_u = np.linspace(0.0, 36.0, 4000)
_y = 0.5 * (np.cos(np.pi * np.sqrt(_u) / CUTOFF) + 1.0)
COS_COEF = np.polynomial.chebyshev.Chebyshev.fit(_u, _y, 9).convert(
    kind=np.polynomial.polynomial.Polynomial).coef.astype(np.float64)


def build_program():
    nc = bacc.Bacc(None, target_bir_lowering=False)
    d = nc.dram_tensor
    # ---------- external inputs ----------
    h16 = d("h16", [BN, ATOM_DIM], F16, kind="ExternalInput")
    zET = d("zET", [32, NZ + 1], F16, kind="ExternalInput")
    eFT = d("eFT", [32, NEL], F16, kind="ExternalInput")
    peW1j = d("peW1j", [32, PHID], F16, kind="ExternalInput")
    peW1k = d("peW1k", [32, PHID], F16, kind="ExternalInput")
    peW1e = d("peW1e", [32, PHID], F16, kind="ExternalInput")
    peW2 = d("peW2", [PHID, PHID], F16, kind="ExternalInput")
    peW3 = d("peW3", [PHID, SCAT], F16, kind="ExternalInput")
    peB1 = d("peB1", [PHID, 1], F32, kind="ExternalInput")
    peB2 = d("peB2", [PHID, 1], F32, kind="ExternalInput")
    peB3 = d("peB3", [128, 1], F32, kind="ExternalInput")
    gW1hj = d("gW1hj", [128, GHID], F16, kind="ExternalInput")
    gW1hk = d("gW1hk", [128, GHID], F16, kind="ExternalInput")
    gW1f = d("gW1f", [97, GHID], F16, kind="ExternalInput")
    gW2a = d("gW2a", [128, GHID], F16, kind="ExternalInput")
    gW2b = d("gW2b", [128, GHID], F16, kind="ExternalInput")
    gW3a = d("gW3a", [128, SCAT], F16, kind="ExternalInput")
    gW3b = d("gW3b", [128, SCAT], F16, kind="ExternalInput")
    gB1 = d("gB1", [128, 2], F32, kind="ExternalInput")
    gB2 = d("gB2", [128, 2], F32, kind="ExternalInput")
    gB3t = d("gB3t", [128, SCAT], F32, kind="ExternalInput")
    oW1 = d("oW1", [SCAT, GHID], F16, kind="ExternalInput")
    oW2a = d("oW2a", [128, ODIM], F16, kind="ExternalInput")
    oW2b = d("oW2b", [128, ODIM], F16, kind="ExternalInput")
    oB1 = d("oB1", [128, 2], F32, kind="ExternalInput")
    oB2 = d("oB2", [128, 1], F32, kind="ExternalInput")
    # per-core path shard data
    rall = d("rall", [128, 3 * WCH], F32, kind="ExternalInput")   # rj|rk|rjk
    cang = d("cang", [128, WCH], F32, kind="ExternalInput")
    idxj = d("idxj", [128, NBLK * 32], I16, kind="ExternalInput")
    idxk = d("idxk", [128, NBLK * 32], I16, kind="ExternalInput")
    # replicated path data (all 45056 slots)
    ctrd = d("ctrd", [128, CHUNKS], F32, kind="ExternalInput")
    idxc = d("idxc", [128, NSG * 64], I16, kind="ExternalInput")
    # ---------- output ----------
    out = d("out", [BN, NEL, ODIM], F32, kind="ExternalOutput")
    # ---------- internal DRAM ----------
    G = d("G", [NCLS_PAD, GROW], F16)
    m_self = d("m_self", [128, WCH * 33], F16)
    m_shared = d("m_shared", [128 * NCORES, WCH * 33], F16, addr_space="Shared")

    with TileContext(nc) as tc:
        with tc.tile_pool(name="c1", bufs=1) as c1, \
             tc.tile_pool(name="wk", bufs=2) as wk, \
             tc.tile_pool(name="gb", bufs=3) as gpool, \
             tc.tile_pool(name="pbig", bufs=2, space="PSUM") as pbig, \
             tc.tile_pool(name="ptr", bufs=2, space="PSUM") as ptr, \
             tc.tile_pool(name="pagg", bufs=2, space="PSUM") as pagg:
            pl3 = ptr

            # ================= setup =================
            def load(t, dram, dt):
                tt = c1.tile(list(dram.shape), dt, tag=t)
                nc.sync.dma_start(out=tt[:], in_=dram[:])
                return tt

            zETt = load("zET", zET, F16)
            eFTt = load("eFT", eFT, F16)
            peW1jt = load("peW1j", peW1j, F16)
            peW1kt = load("peW1k", peW1k, F16)
            peW1et = load("peW1e", peW1e, F16)
            peW2t = load("peW2", peW2, F16)
            peW3t = load("peW3", peW3, F16)
            peB1t = load("peB1", peB1, F32)
            peB2t = load("peB2", peB2, F32)
            peB3t = load("peB3", peB3, F32)
            gW1hjt = load("gW1hj", gW1hj, F16)
            gW1hkt = load("gW1hk", gW1hk, F16)
            gW1ft = load("gW1f", gW1f, F16)
            gW2at = load("gW2a", gW2a, F16)
            gW2bt = load("gW2b", gW2b, F16)
            gW3at = load("gW3a", gW3a, F16)
            gW3bt = load("gW3b", gW3b, F16)
            gB1t = load("gB1", gB1, F32)
            gB2t = load("gB2", gB2, F32)
            gB3tt = load("gB3t", gB3t, F32)
            oW1t = load("oW1", oW1, F16)
            oW2at = load("oW2a", oW2a, F16)
            oW2bt = load("oW2b", oW2b, F16)
            oB1t = load("oB1", oB1, F32)
            oB2t = load("oB2", oB2, F32)
            rallt = load("rall", rall, F32)
            cangt = load("cang", cang, F32)
            idxjt = load("idxj", idxj, I16)
            idxkt = load("idxk", idxk, I16)
            ctrt = load("ctrd", ctrd, F32)
            idxct = load("idxc", idxc, I16)

            # identity (fp16) for PE transposes + window iota tiles (f32)
            iota_row = c1.tile([128, 128], F32, tag="iota_row")
            nc.gpsimd.iota(iota_row[:], pattern=[[1, 128]], base=0,
                           channel_multiplier=0,
                           allow_small_or_imprecise_dtypes=True)
            iota_col = c1.tile([128, 1], F32, tag="iota_col")
            nc.gpsimd.iota(iota_col[:], pattern=[[0, 1]], base=0,
                           channel_multiplier=1,
                           allow_small_or_imprecise_dtypes=True)
            ident = c1.tile([128, 128], F16, tag="ident")
            nc.vector.tensor_tensor(ident[:], iota_row[:],
                                    iota_col[:].broadcast_to([128, 128]),
                                    ALU.is_equal)
            iwtmp = c1.tile([128, 128], F32, tag="iwtmp")
            iwin = []
            for w in range(NWIN):
                nc.gpsimd.iota(iwtmp[:], pattern=[[1, 128]], base=128 * w,
                               channel_multiplier=0,
                               allow_small_or_imprecise_dtypes=True)
                t = c1.tile([128, 128], F16, tag=f"iwin{w}")
                nc.vector.tensor_copy(t[:], iwtmp[:])
                iwin.append(t)
            # rbf centers tile (128, 32) f32: k * CUTOFF/(RBF-1)
            cent = c1.tile([128, RBF], F32, tag="cent")
            nc.gpsimd.iota(cent[:], pattern=[[1, RBF]], base=0,
                           channel_multiplier=0,
                           allow_small_or_imprecise_dtypes=True)
            nc.vector.tensor_scalar_mul(cent[:], cent[:], float(RBF_W))

            # ---- tiny matmuls: tjT/tkT (128,100) and vT (128, NEL) ----
            tjp = pl3.tile([128, NZ], F32, tag="tr")
            nc.tensor.matmul(tjp[:], peW1jt[:], zETt[:, 1:NZ + 1],
                             start=True, stop=True)
            tjTs = c1.tile([128, NZ], F32, tag="tjT")
            nc.vector.tensor_scalar_add(tjTs[:], tjp[:], peB1t[:])
            tkp = pl3.tile([128, NZ], F32, tag="tr")
            nc.tensor.matmul(tkp[:], peW1kt[:], zETt[:, 1:NZ + 1],
                             start=True, stop=True)
            tkTs = c1.tile([128, NZ], F32, tag="tkT")
            nc.vector.tensor_copy(tkTs[:], tkp[:])
            vp = pl3.tile([128, NEL], F32, tag="tr")
            nc.tensor.matmul(vp[:], peW1et[:], eFTt[:],
                             start=True, stop=True)
            vTs = c1.tile([128, NEL], F32, tag="vT")
            nc.vector.tensor_copy(vTs[:], vp[:])

            # ---- u grid (128, NCLS_PAD) fp16 ----
            ugrid = c1.tile([128, NCLS_PAD], F16, tag="ugrid")
            nc.vector.memset(ugrid[:, NCLS:NCLS_PAD], 0.0)
            for k in range(5):
                zsl = slice(20 * k, 20 * (k + 1))
                nc.vector.tensor_tensor(
                    ugrid[:, 2000 * k:2000 * (k + 1)].rearrange(
                        "p (a b) -> p a b", b=NZ),
                    tjTs[:, zsl].unsqueeze(2).broadcast_to([128, 20, NZ]),
                    tkTs[:].unsqueeze(1).broadcast_to([128, 20, NZ]),
                    ALU.add)

            # ================= phase A: g_geom + m for own shard ============
            # rbf features + cutoff weights for own 5632 paths
            rmin = wk.tile([128, 3 * WCH], F32, tag="rmin")
            nc.vector.tensor_scalar_min(rmin[:], rallt[:], float(CUTOFF))
            # cutoff polynomial in u = r*r
            upoly = wk.tile([128, 3 * WCH], F32, tag="upoly")
            nc.vector.tensor_tensor(upoly[:], rallt[:], rallt[:], ALU.mult)
            y0 = wk.tile([128, 3 * WCH], F32, tag="y0")
            y1 = wk.tile([128, 3 * WCH], F32, tag="y1")
            nc.vector.memset(y0[:], float(COS_COEF[9]))
            cur, nxt = y0, y1
            for k in range(8, -1, -1):
                nc.vector.tensor_tensor(nxt[:], cur[:], upoly[:], ALU.mult)
                nc.vector.tensor_scalar_add(nxt[:], nxt[:], float(COS_COEF[k]))
                cur, nxt = nxt, cur
            mask = wk.tile([128, 3 * WCH], F32, tag="mask")
            nc.vector.tensor_scalar(mask[:], rallt[:], float(CUTOFF), None,
                                    ALU.is_lt)
            nc.vector.tensor_tensor(cur[:], cur[:], mask[:], ALU.mult)
            wpr = c1.tile([128, WCH], F32, tag="wpr")
            nc.vector.tensor_tensor(wpr[:], cur[:, 0:WCH], cur[:, WCH:2 * WCH],
                                    ALU.mult)
            nc.vector.tensor_tensor(wpr[:], wpr[:], cur[:, 2 * WCH:3 * WCH],
                                    ALU.mult)

            # rbf feature pack, path-major: (128, WCH, 112)
            feat = c1.tile([128, WCH, 112], F16, tag="feat")
            for kind in range(3):
                rsl = rmin[:, kind * WCH:(kind + 1) * WCH]
                dmt = wk.tile([128, WCH, RBF], F32, tag="dmt")
                nc.vector.tensor_tensor(
                    dmt[:], cent[:].unsqueeze(1).broadcast_to([128, WCH, RBF]),
                    rsl.unsqueeze(2).broadcast_to([128, WCH, RBF]),
                    ALU.subtract)
                nc.vector.tensor_tensor(dmt[:], dmt[:], dmt[:], ALU.mult)
                nc.scalar.activation(feat[:, :, kind * RBF:(kind + 1) * RBF],
                                     dmt[:], AF.Exp, scale=float(RBF_COEF))
            nc.vector.tensor_copy(feat[:, :, 96:97], cangt[:].unsqueeze(2))

            m_all = c1.tile([128, WCH, 33], F16, tag="m_all")
            nc.vector.tensor_copy(m_all[:, :, 32:33], wpr[:].unsqueeze(2))

            for b in range(NBLK):
                hjT = gpool.tile([128, 1, GBLK], F16, tag="hjT")
                nc.gpsimd.dma_gather(hjT[:], h16[:],
                                     idxjt[:, 32 * b:32 * (b + 1)],
                                     GBLK, GBLK, ATOM_DIM, transpose=True)
                hkT = gpool.tile([128, 1, GBLK], F16, tag="hkT")
                nc.gpsimd.dma_gather(hkT[:], h16[:],
                                     idxkt[:, 32 * b:32 * (b + 1)],
                                     GBLK, GBLK, ATOM_DIM, transpose=True)
                fT = wk.tile([97, GBLK], F16, tag="fT")
                for c in range(4):
                    tp = pagg.tile([97, 128], F16, tag="agg")
                    nc.tensor.transpose(tp[:], feat[:, 4 * b + c, 0:97],
                                        ident[:])
                    nc.vector.tensor_copy(fT[:, 128 * c:128 * (c + 1)], tp[:])
                # L1: H1T (256, 512) = silu(W1^T X^T + b1)
                h1a = wk.tile([128, GBLK], F16, tag="h1a")
                h1b = wk.tile([128, GBLK], F16, tag="h1b")
                for half, h1 in ((0, h1a), (1, h1b)):
                    ps = pagg.tile([128, GBLK], F32, tag="agg")
                    sl = slice(128 * half, 128 * (half + 1))
                    nc.tensor.matmul(ps[:], gW1hjt[:, sl], hjT[:, 0, :], start=True,
                                     stop=False)
                    nc.tensor.matmul(ps[:], gW1hkt[:, sl], hkT[:, 0, :], start=False,
                                     stop=False)
                    nc.tensor.matmul(ps[:], gW1ft[:, sl], fT[:], start=False,
                                     stop=True)
                    nc.scalar.activation(h1[:], ps[:], AF.Silu,
                                         bias=gB1t[:, half:half + 1])
                # L2
                h2a = wk.tile([128, GBLK], F16, tag="h2a")
                h2b = wk.tile([128, GBLK], F16, tag="h2b")
                for half, h2 in ((0, h2a), (1, h2b)):
                    ps = pagg.tile([128, GBLK], F32, tag="agg")
                    sl = slice(128 * half, 128 * (half + 1))
                    nc.tensor.matmul(ps[:], gW2at[:, sl], h1a[:], start=True,
                                     stop=False)
                    nc.tensor.matmul(ps[:], gW2bt[:, sl], h1b[:], start=False,
                                     stop=True)
                    nc.scalar.activation(h2[:], ps[:], AF.Silu,
                                         bias=gB2t[:, half:half + 1])
                # L3 path-major + bias + cutoff weight -> m_all
                for c in range(4):
                    cc = 4 * b + c
                    pm = pagg.tile([128, SCAT], F32, tag="agg")
                    sl = slice(128 * c, 128 * (c + 1))
                    nc.tensor.matmul(pm[:], h2a[:, sl], gW3at[:], start=True,
                                     stop=False)
                    nc.tensor.matmul(pm[:], h2b[:, sl], gW3bt[:], start=False,
                                     stop=True)
                    t32 = wk.tile([128, SCAT], F32, tag="t32")
                    nc.vector.tensor_tensor(t32[:], pm[:], gB3tt[:], ALU.add)
                    nc.vector.tensor_scalar_mul(m_all[:, cc, 0:SCAT], t32[:],
                                                wpr[:, cc:cc + 1])
            nc.sync.dma_start(out=m_self[:], in_=m_all[:])
            # ================= phase T: class table ====================
            # e-groups stacked 4-wide on partitions for the L3 drain
            EGROUPS = [(0, 4), (4, 4), (8, 2)]
            for e0, Q in EGROUPS:
                for cc in range(NCLS_PAD // CW):
                    usl = ugrid[:, CW * cc:CW * (cc + 1)]
                    h2s = []
                    for q in range(Q):
                        h1 = wk.tile([128, CW], F16, tag="th1")
                        nc.scalar.activation(h1[:], usl, AF.Silu,
                                             bias=vTs[:, e0 + q:e0 + q + 1])
                        h2 = c1.tile([128, CW], F16, tag=f"th2q{q}")
                        for half in range(2):
                            ps = pbig.tile([128, 1024], F32, tag="big")
                            for p2 in range(2):
                                sl = slice(1024 * half + 512 * p2,
                                           1024 * half + 512 * (p2 + 1))
                                nc.tensor.matmul(ps[:, 512 * p2:512 * (p2 + 1)],
                                                 peW2t[:], h1[:, sl],
                                                 start=True, stop=True)
                            nc.scalar.activation(
                                h2[:, 1024 * half:1024 * (half + 1)], ps[:],
                                AF.Silu, bias=peB2t[:])
                        h2s.append(h2)
                    gbt4 = wk.tile([128, CW], F16, tag="tgb")
                    for piece in range(4):
                        ps3 = pl3.tile([128, 512], F32, tag="tr")
                        sl = slice(512 * piece, 512 * (piece + 1))
                        for q in range(Q):
                            nc.tensor.matmul(ps3[32 * q:32 * (q + 1), :],
                                             peW3t[:], h2s[q][:, sl],
                                             start=True, stop=True,
                                             tile_position=(0, 32 * q),
                                             skip_group_check=True)
                        nc.vector.tensor_scalar_add(
                            gbt4[0:32 * Q, sl], ps3[0:32 * Q, :], peB3t[0:32 * Q, :])
                    for q in range(Q):
                        trb = ptr.tile([128, 512], F16, tag="tr")
                        qq = slice(32 * q, 32 * (q + 1))
                        for t in range(16):
                            nc.tensor.transpose(
                                trb[:, 32 * t:32 * (t + 1)],
                                gbt4[qq, 128 * t:128 * (t + 1)],
                                ident[qq, qq],
                                tile_position=(32 * q, 0))
                        gst = wk.tile([128, 512], F16, tag="gst")
                        nc.vector.tensor_copy(gst[:], trb[:])
                        e = e0 + q
                        nc.sync.dma_start(
                            out=G[CW * cc:CW * (cc + 1),
                                  32 * e:32 * (e + 1)].rearrange(
                                      "(t p) s -> p t s", p=128),
                            in_=gst[:].rearrange("p (t s) -> p t s", s=32))

            nc.gpsimd.collective_compute(
                "AllGather", ALU.bypass,
                replica_groups=[list(range(NCORES))],
                ins=[m_self[:]], outs=[m_shared[:]])
            m_full = c1.tile([128, CHUNKS, 33], F16, tag="m_full")
            nc.gpsimd.dma_start(
                out=m_full[:].rearrange("p (r c) s -> p r c s", r=NCORES),
                in_=m_shared.rearrange("(r p) (c s) -> p r c s", p=128, s=33))

            # ============ phase S + O: gather/contrib/scatter + out MLP ====
            tc.tile_set_cur_wait(0.185)
            aggps = [None] * NWIN
            ohts = {}
            for sg in range(NSG):
                gbuf = gpool.tile([128, SG, GROW], F16, tag="gbuf")
                nc.gpsimd.dma_gather(gbuf[:], G[:],
                                     idxct[:, 64 * sg:64 * (sg + 1)],
                                     SG * 128, SG * 128, GROW)
                contrib = wk.tile([128, SG, 354], F16, tag="contrib")
                nc.vector.tensor_tensor(
                    contrib[:, :, 0:320].rearrange(
                        "p c (e s) -> p c e s", s=32),
                    gbuf[:, :, 0:320].rearrange("p c (e s) -> p c e s", s=32),
                    m_full[:, SG * sg:SG * (sg + 1), 0:SCAT].unsqueeze(
                        2).broadcast_to([128, SG, NEL, SCAT]),
                    ALU.mult)
                nc.scalar.activation(
                    contrib[:, :, 321:354],
                    m_full[:, SG * sg:SG * (sg + 1), 0:33], AF.Copy)
                for c in range(SG):
                    C = SG * sg + c
                    w = C // WCH
                    t = C % WCH
                    oht = wk.tile([128, 128], F16, tag="oht")
                    nc.vector.tensor_scalar(oht[:], iwin[w][:],
                                            ctrt[:, C:C + 1], None,
                                            ALU.is_equal)
                    oh = oht[:]
                    if t == 0:
                        agg_t = pagg.tile([128, 354], F32, tag="agg")
                        aggps[w] = agg_t
                    nc.tensor.matmul(aggps[w][:], oh, contrib[:, c, :],
                                     start=(t == 0), stop=(t == WCH - 1),
                                     skip_group_check=True)
                    if t == WCH - 1:
                        _emit_window_out(nc, wk, pbig, pl3, ptr, aggps, w,
                                         ident, oW1t, oW2at, oW2bt, oB1t,
                                         oB2t, out)
    nc.compile()
    return nc


def _emit_window_out(nc, wk, pbig, pl3, ptr, aggps, w, ident, oW1t, oW2at,
                     oW2bt, oB1t, oB2t, out):
    """Normalize window w's agg and run the output MLP, write DRAM."""
    agg = aggps[w]
    nrm = wk.tile([128, 1], F32, tag="nrm")
    nc.vector.tensor_scalar_max(nrm[:], agg[:, 353:354], float(EPS))
    rin = wk.tile([128, 1], F32, tag="rin")
    nc.vector.reciprocal(rin[:], nrm[:])
    aggn = wk.tile([128, 320], F16, tag="aggn")
    nc.vector.tensor_scalar_mul(aggn[:], agg[:, 0:320], rin[:])
    slab = wk.tile([128, NEL, ODIM], F32, tag="slab")
    EG = 5
    for g in range(NEL // EG):
        xts = wk.tile([SCAT, EG, 128], F16, tag="xts")
        for q in range(EG):
            e = EG * g + q
            xt = ptr.tile([SCAT, 128], F16, tag="tr")
            nc.tensor.transpose(xt[:], aggn[:, 32 * e:32 * (e + 1)], ident[:])
            nc.scalar.activation(xts[:, q, :], xt[:], AF.Copy)
        h1a = wk.tile([128, EG * 128], F16, tag="oh1a")
        h1b = wk.tile([128, EG * 128], F16, tag="oh1b")
        for half, h1 in ((0, h1a), (1, h1b)):
            ps = pbig.tile([128, EG * 128], F32, tag="big")
            sl = slice(128 * half, 128 * (half + 1))
            xr = xts[:].rearrange("p q a -> p (q a)")
            nc.tensor.matmul(ps[:, 0:512], oW1t[:, sl], xr[:, 0:512],
                             start=True, stop=True, skip_group_check=True)
            nc.tensor.matmul(ps[:, 512:640], oW1t[:, sl], xr[:, 512:640],
                             start=True, stop=True, skip_group_check=True)
            nc.scalar.activation(h1[:], ps[:], AF.Silu,
                                 bias=oB1t[:, half:half + 1])
        po = pbig.tile([128, EG * 128], F32, tag="big")
        nc.tensor.matmul(po[:, 0:512], oW2at[:], h1a[:, 0:512],
                         start=True, stop=False, skip_group_check=True)
        nc.tensor.matmul(po[:, 0:512], oW2bt[:], h1b[:, 0:512],
                         start=False, stop=True, skip_group_check=True)
        nc.tensor.matmul(po[:, 512:640], oW2at[:], h1a[:, 512:640],
                         start=True, stop=False, skip_group_check=True)
        nc.tensor.matmul(po[:, 512:640], oW2bt[:], h1b[:, 512:640],
                         start=False, stop=True, skip_group_check=True)
        ot16 = wk.tile([128, EG * 128], F16, tag="ot16")
        nc.vector.tensor_scalar_add(ot16[:], po[:], oB2t[:])
        for q in range(EG):
            e = EG * g + q
            tro = ptr.tile([128, 128], F16, tag="tr")
            nc.tensor.transpose(tro[:], ot16[:, 128 * q:128 * (q + 1)],
                                ident[:])
            nc.scalar.activation(slab[:, e, :], tro[:], AF.Copy)
    nc.sync.dma_start(out=out[128 * w:128 * (w + 1)], in_=slab[:])


# =================== host side ===================
_PROG = None
LAST_RESULT = None


def _wrap_idx(a, gs):
    """dma_gather index layout: groups of gs, 16-wrap, replicated x8."""
    a = np.asarray(a, np.int64)
    ng = len(a) // gs
    blk = a.reshape(ng, gs // 16, 16)           # [g, s, p] = a[g, s*16+p]
    outc = blk.transpose(2, 0, 1).reshape(16, ng * (gs // 16))
    return np.tile(outc, (8, 1)).astype(np.int16)


def _colchunk(a):
    return np.ascontiguousarray(np.asarray(a).reshape(-1, 128).T)


def _prepare(inputs):
    h_flat = np.asarray(inputs["h_flat"], np.float32)
    z_flat = np.asarray(inputs["z_flat"], np.int32)
    e_feat = np.asarray(inputs["e_feat"], np.float32)
    pc = np.asarray(inputs["path_center"], np.int64)
    pj = np.asarray(inputs["path_j"], np.int64)
    pk = np.asarray(inputs["path_k"], np.int64)
    r0j = np.asarray(inputs["path_r0j"], np.float32)
    r0k = np.asarray(inputs["path_r0k"], np.float32)
    rjk = np.asarray(inputs["path_rjk"], np.float32)
    ca = np.asarray(inputs["path_cosangle"], np.float32)
    bsz = int(inputs["bsz"])
    n_atoms = int(inputs["n_atoms"])
    assert bsz * n_atoms == BN and pc.shape[0] == P

    # ---- sort by center, bucket into 8 windows, pad to SLOTS each ----
    order = np.argsort(pc, kind="stable")
    win = pc[order] // 128
    idx_pad = np.zeros(PPAD, np.int64)
    valid = np.zeros(PPAD, bool)
    for w in range(NWIN):
        sel = order[win == w]
        assert len(sel) <= SLOTS, f"window {w} overflow: {len(sel)}"
        idx_pad[w * SLOTS:w * SLOTS + len(sel)] = sel
        valid[w * SLOTS:w * SLOTS + len(sel)] = True

    def takef(a, fill):
        o = np.full(PPAD, fill, np.float32)
        o[valid] = a[idx_pad[valid]]
        return o

    ctr = np.where(valid, pc[idx_pad],
                   (np.arange(PPAD) // SLOTS) * 128).astype(np.float32)
    jj = np.where(valid, pj[idx_pad], 0)
    kk = np.where(valid, pk[idx_pad], 0)
    rj_p = takef(r0j, 6.0)
    rk_p = takef(r0k, 6.0)
    rjk_p = takef(rjk, 6.0)
    ca_p = takef(ca, 0.0)
    cls = ((z_flat[jj] - 1).astype(np.int64) * NZ
           + (z_flat[kk] - 1)).astype(np.int64)
    np.clip(cls, 0, NCLS - 1, out=cls)

    # ---- weights / common tensors ----
    f16 = lambda a: np.ascontiguousarray(a, np.float32).astype(np.float16)
    f32c = lambda a, shape=None: np.ascontiguousarray(
        np.asarray(a, np.float32).reshape(shape) if shape else
        np.asarray(a, np.float32))
    g_W1 = np.asarray(inputs["g_W1"], np.float32)
    g_W2 = np.asarray(inputs["g_W2"], np.float32)
    g_W3 = np.asarray(inputs["g_W3"], np.float32)
    o_W2 = np.asarray(inputs["o_W2"], np.float32)
    z_emb = np.asarray(inputs["z_emb"], np.float32)
    common = dict(
        h16=f16(h_flat),
        zET=f16(z_emb.T),
        peW1j=f16(np.asarray(inputs["pe_W1"], np.float32)[0:32]),
        peW1k=f16(np.asarray(inputs["pe_W1"], np.float32)[32:64]),
        peW1e=f16(np.asarray(inputs["pe_W1"], np.float32)[64:96]),
        peW2=f16(inputs["pe_W2"]),
        peW3=f16(inputs["pe_W3"]),
        peB1=f32c(inputs["pe_b1"], (PHID, 1)),
        peB2=f32c(inputs["pe_b2"], (PHID, 1)),
        peB3=f32c(np.tile(np.asarray(inputs["pe_b3"], np.float32), 4), (128, 1)),
        gW1hj=f16(g_W1[0:128]),
        gW1hk=f16(g_W1[128:256]),
        gW1f=f16(g_W1[256:353]),
        gW2a=f16(g_W2[0:128]),
        gW2b=f16(g_W2[128:256]),
        gW3a=f16(g_W3[0:128]),
        gW3b=f16(g_W3[128:256]),
        gB1=f32c(np.asarray(inputs["g_b1"]).reshape(2, 128).T),
        gB2=f32c(np.asarray(inputs["g_b2"]).reshape(2, 128).T),
        gB3t=f32c(np.broadcast_to(np.asarray(inputs["g_b3"], np.float32),
                                  (128, SCAT))),
        oW1=f16(inputs["o_W1"]),
        oW2a=f16(o_W2[0:128]),
        oW2b=f16(o_W2[128:256]),
        oB1=f32c(np.asarray(inputs["o_b1"]).reshape(2, 128).T),
        oB2=f32c(inputs["o_b2"], (ODIM, 1)),
        ctrd=_colchunk(ctr),
        idxc=_wrap_idx(cls, SG * 128),
    )

    in_maps = []
    for i in range(NCORES):
        sl = slice(SLOTS * i, SLOTS * (i + 1))
        rall_i = np.concatenate(
            [_colchunk(rj_p[sl]), _colchunk(rk_p[sl]), _colchunk(rjk_p[sl])],
            axis=1)
        m = dict(common)
        m.update(
            eFT=f16(e_feat[NEL * i:NEL * (i + 1)].T),
            rall=np.ascontiguousarray(rall_i),
            cang=_colchunk(ca_p[sl]),
            idxj=_wrap_idx(jj[sl], GBLK),
            idxk=_wrap_idx(kk[sl], GBLK),
        )
        in_maps.append(m)
    return in_maps


def _run_device(in_maps):
    global _PROG, LAST_RESULT
    if _PROG is None:
        _PROG = build_program()
    from concourse.bass_utils import run_bass_kernel_spmd
    res = run_bass_kernel_spmd(_PROG, in_maps, list(range(NCORES)))
    LAST_RESULT = res
    return [np.asarray(res.results[i]["out"]) for i in range(NCORES)]


def _run_subprocess(in_maps):
    """Fresh-process retry: a wedged NRT/PJRT client can only be recovered by
    a process restart (stale collective-comm state flips clean/dirty per
    process)."""
    import pickle, subprocess, tempfile, time as _time
    d = tempfile.mkdtemp()
    fin = os.path.join(d, "in.pkl")
    fout = os.path.join(d, "out.npz")
    with open(fin, "wb") as f:
        pickle.dump(in_maps, f)
    last = None
    for attempt in range(4):
        p = subprocess.run([sys.executable, os.path.abspath(__file__),
                            "--worker", fin, fout],
                           capture_output=True, text=True, timeout=1800)
        if p.returncode == 0 and os.path.exists(fout):
            z = np.load(fout)
            return [z[f"out{i}"] for i in range(NCORES)]
        last = p.stderr[-2000:]
        _time.sleep(5)
    raise RuntimeError(f"device run failed after retries: {last}")


def kernel(**inputs):
    bsz = int(inputs["bsz"])
    n_atoms = int(inputs["n_atoms"])
    in_maps = _prepare(inputs)
    try:
        outs = _run_device(in_maps)
    except Exception:
        outs = _run_subprocess(in_maps)

    full = np.empty((BN, NE, ODIM), np.float32)
    for i in range(NCORES):
        full[:, NEL * i:NEL * (i + 1), :] = outs[i]
    return full.reshape(bsz, n_atoms, NE, ODIM)


if __name__ == "__main__":
    if len(sys.argv) >= 4 and sys.argv[1] == "--worker":
        import pickle
        with open(sys.argv[2], "rb") as f:
            _maps = pickle.load(f)
        _outs = _run_device(_maps)
        np.savez(sys.argv[3], **{f"out{i}": o for i, o in enumerate(_outs)})
        print("worker ok")
    else:
        build_program()
        print("build ok")
